# revision 8
# baseline (speedup 1.0000x reference)
"""Trainium2 Bass kernel for CrossAttention.

  q = img @ Wq + bq            [N_img, D]
  k = text @ Wk + bk           [N_txt, D]
  v = text @ Wv + bv           [N_txt, D]
  attn = softmax(q @ k.T, axis=1)
  out = (attn @ v) @ Wo + bo   [N_img, D]

Sharding: img rows split across 8 cores (row-parallel attention with
replicated weights). The k/v projections are 8-way sharded too — each
core computes a 1/8 slice (kT by output-feature rows, v by text rows)
and the full tensors are AllGathered while the img transpose + q
projection run. The per-core slice is selected host-side by feeding
each core its own Wk/bk column slice and text row slice, so the SPMD
program is identical across cores.

All matmuls run as float32r (full-rate truncated-fp32 streaming mode).
Softmax normalization is deferred: exp(s - max) stays unnormalized and
the 1/rowsum scale is applied per-partition during the final
output-projection PSUM eviction (exact because rows of attn scale
linearly through attn@v and @Wo; bo is added after the scale).

SBUF pools are stack-allocated per side; the overlapping phase chain
textT -> imgT -> qT -> attn -> attnT -> avT alternates sides so every
release is top-of-stack on its side.
"""

import os
import sys

sys.path.insert(0, "/opt/trn_rl_repo")

import numpy as np

import concourse.bass as bass
import concourse.tile as tile
from concourse import bacc, mybir
from concourse.bass_utils import run_bass_kernel_spmd
from concourse.masks import make_identity

F32 = mybir.dt.float32
F32R = mybir.dt.float32r
AF = mybir.ActivationFunctionType
P = 128
NCORES = 8

SMALL = bool(int(os.environ.get("BASS_SMALL", "0")))
if SMALL:
    N_IMG, D, N_TXT, DT, NB = 2048, 1024, 1024, 512, 256
else:
    N_IMG, D, N_TXT, DT, NB = 8192, 2048, 2048, 1024, 512

M = N_IMG // NCORES     # img rows per core
SL = N_TXT // NCORES    # text rows / kT feature-rows per core slice
SLD = D // NCORES       # Wk columns per core slice
DC = D // P             # chunks of the model dim
TC = N_TXT // P         # chunks of the text-token dim
DTC = DT // P           # chunks of the text-feature dim
MB = M // P             # img row blocks per core
SLB = SLD // P          # 128-blocks in the per-core kT slice
SLT = SL // P           # 128-blocks in the per-core text slice


def build():
    nc = bacc.Bacc("TRN2", target_bir_lowering=False, debug=False,
                   num_devices=NCORES)
    img = nc.declare_dram_parameter("img", [M, D], F32, isOutput=False)
    text = nc.declare_dram_parameter("text_embedding", [N_TXT, DT], F32,
                                     isOutput=False)
    text_sl = nc.declare_dram_parameter("text_slice", [SL, DT], F32,
                                        isOutput=False)
    Wq = nc.declare_dram_parameter("Wq", [D, D], F32, isOutput=False)
    bq = nc.declare_dram_parameter("bq", [D], F32, isOutput=False)
    Wk = nc.declare_dram_parameter("Wk_slice", [DT, SLD], F32, isOutput=False)
    bk = nc.declare_dram_parameter("bk_slice", [SLD], F32, isOutput=False)
    Wv = nc.declare_dram_parameter("Wv", [DT, D], F32, isOutput=False)
    bv = nc.declare_dram_parameter("bv", [D], F32, isOutput=False)
    Wo = nc.declare_dram_parameter("Wo", [D, D], F32, isOutput=False)
    bo = nc.declare_dram_parameter("bo", [D], F32, isOutput=False)
    out = nc.declare_dram_parameter("out", [M, D], F32, isOutput=True)

    def bcast_ap(handle, n):
        ap = handle.ap()
        return bass.AP(tensor=ap.tensor, offset=ap.offset,
                       ap=[[0, P]] + list(ap.ap))[:, :n]

    with tile.TileContext(nc) as tc:
        consts = tc.alloc_tile_pool(name="consts", bufs=1, side="left")
        tpsum = tc.alloc_tile_pool(name="tpsum", bufs=3, space="PSUM",
                                   side="left")
        mpsum = tc.alloc_tile_pool(name="mpsum", bufs=4, space="PSUM",
                                   side="left")
        dram = tc.alloc_tile_pool(name="dram", bufs=1, space="DRAM")

        kT_slice = dram.tile([SLD, N_TXT], F32, tag="kts")
        v_slice = dram.tile([SL, D], F32, tag="vs")
        kT_gath = dram.tile([D, N_TXT], F32, tag="ktg", addr_space="Shared")
        v_gath = dram.tile([N_TXT, D], F32, tag="vg", addr_space="Shared")

        ident = consts.tile([P, P], F32, tag="ident")
        make_identity(nc, ident)
        bq_t = consts.tile([P, DC], F32, tag="bqt")
        nc.gpsimd.dma_start(out=bq_t, in_=bq.rearrange("(c p) -> p c", p=P))
        bk_t = consts.tile([P, SLB], F32, tag="bkt")
        nc.gpsimd.dma_start(out=bk_t, in_=bk.rearrange("(c p) -> p c", p=P))
        negmax = consts.tile([P, MB], F32, tag="negmax")
        rsum = consts.tile([P, MB], F32, tag="rsum")
        recip = consts.tile([P, MB], F32, tag="recip")

        # ---- phase 0: textT[c][dt, t] = text.T, via PE transposes ----
        textT_pool = tc.alloc_tile_pool(name="textT", bufs=1, side="left")
        textTs_pool = tc.alloc_tile_pool(name="textTs", bufs=1, side="left")
        bvp = tc.alloc_tile_pool(name="bvp", bufs=1, side="left")
        bv_bc = bvp.tile([P, D], F32, tag="bvb")
        nc.gpsimd.dma_start(out=bv_bc, in_=bcast_ap(bv, D))
        wv_pool = tc.alloc_tile_pool(name="wv", bufs=3, side="left")
        vstage = tc.alloc_tile_pool(name="vstage", bufs=3, side="left")
        tstream = tc.alloc_tile_pool(name="tstream", bufs=3, side="left")
        textT = [textT_pool.tile([P, N_TXT], F32R, tag=f"tT{c}", name=f"tT{c}")
                 for c in range(DTC)]
        for j in range(TC):
            tr = tstream.tile([P, DT], F32, tag="trow")
            nc.sync.dma_start(out=tr, in_=text[j * P:(j + 1) * P, :])
            for c in range(DTC):
                ps = tpsum.tile([P, P], F32, tag="tps")
                nc.tensor.transpose(ps, tr[:, c * P:(c + 1) * P], ident)
                nc.vector.tensor_copy(textT[c][:, j * P:(j + 1) * P], ps)
        # textTs[c][dt, t_local]: transpose of this core's text row slice
        textTs = [textTs_pool.tile([P, SL], F32R, tag=f"tS{c}", name=f"tS{c}")
                  for c in range(DTC)]
        for j in range(SLT):
            tr = tstream.tile([P, DT], F32, tag="trow")
            nc.sync.dma_start(out=tr, in_=text_sl[j * P:(j + 1) * P, :])
            for c in range(DTC):
                ps = tpsum.tile([P, P], F32, tag="tps")
                nc.tensor.transpose(ps, tr[:, c * P:(c + 1) * P], ident)
                nc.vector.tensor_copy(textTs[c][:, j * P:(j + 1) * P], ps)
        tstream.release()

        # ---- phase 1a: kT slice = (text @ Wk_slice + bk_slice).T ----
        wk_pool = tc.alloc_tile_pool(name="wk", bufs=2, side="left")
        kstage = tc.alloc_tile_pool(name="kstage", bufs=3, side="left")
        for do in range(SLB):
            wk = wk_pool.tile([P, DTC, P], F32R, tag="wk")
            nc.sync.dma_start(
                out=wk,
                in_=Wk[:, do * P:(do + 1) * P]
                .rearrange("(c p) n -> p c n", p=P).bitcast(F32R))
            for tb in range(N_TXT // NB):
                ps = mpsum.tile([P, NB], F32, tag="mps")
                for c in range(DTC):
                    nc.tensor.matmul(
                        ps, wk[:, c, :],
                        textT[c][:, tb * NB:(tb + 1) * NB],
                        start=(c == 0), stop=(c == DTC - 1))
                ks = kstage.tile([P, NB], F32, tag="ks")
                nc.scalar.activation(ks, ps, AF.Identity,
                                     bias=bk_t[:, do:do + 1], scale=1.0)
                nc.sync.dma_start(
                    out=kT_slice[do * P:(do + 1) * P, tb * NB:(tb + 1) * NB],
                    in_=ks)
        nc.gpsimd.collective_compute(
            "AllGather", mybir.AluOpType.bypass,
            replica_groups=[list(range(NCORES))],
            ins=[kT_slice.opt()], outs=[kT_gath.opt()])
        kstage.release()
        wk_pool.release()

        # ---- phase 1b: v slice = text_slice @ Wv + bv (row-major) ----
        for ob in range(D // NB):
            wv = wv_pool.tile([P, DTC, NB], F32R, tag="wv")
            nc.sync.dma_start(
                out=wv,
                in_=Wv[:, ob * NB:(ob + 1) * NB]
                .rearrange("(c p) n -> p c n", p=P).bitcast(F32R))
            for j in range(SLT):
                ps = mpsum.tile([P, NB], F32, tag="mps")
                for c in range(DTC):
                    nc.tensor.matmul(
                        ps, textTs[c][:, j * P:(j + 1) * P],
                        wv[:, c, :],
                        start=(c == 0), stop=(c == DTC - 1))
                vs = vstage.tile([P, NB], F32, tag="vs")
                nc.vector.tensor_add(vs, ps, bv_bc[:, ob * NB:(ob + 1) * NB])
                nc.sync.dma_start(
                    out=v_slice[j * P:(j + 1) * P, ob * NB:(ob + 1) * NB],
                    in_=vs)
        vstage.release()
        wv_pool.release()
        bvp.release()
        textTs_pool.release()
        textT_pool.release()

        # ---- phase 2: imgT[c][d, i] = img.T ----
        imgT_pool = tc.alloc_tile_pool(name="imgT", bufs=1, side="right")
        istream = tc.alloc_tile_pool(name="istream", bufs=3, side="right")
        imgT = [imgT_pool.tile([P, M], F32R, tag=f"iT{c}", name=f"iT{c}")
                for c in range(DC)]
        for rb in range(MB):
            ir = istream.tile([P, D], F32, tag="irow")
            nc.sync.dma_start(out=ir, in_=img[rb * P:(rb + 1) * P, :])
            for c in range(DC):
                ps = tpsum.tile([P, P], F32, tag="tps")
                nc.tensor.transpose(ps, ir[:, c * P:(c + 1) * P], ident)
                nc.vector.tensor_copy(imgT[c][:, rb * P:(rb + 1) * P], ps)
        istream.release()

        # ---- phase 3: qT[do][d_out, i] = (img @ Wq + bq).T, SBUF-resident ----
        qT_pool = tc.alloc_tile_pool(name="qT", bufs=1, side="left")
        wq_pool = tc.alloc_tile_pool(name="wq", bufs=2, side="left")
        qT = [qT_pool.tile([P, M], F32R, tag=f"qT{c}", name=f"qT{c}")
              for c in range(DC)]
        for do in range(DC):
            wq = wq_pool.tile([P, DC, P], F32R, tag="wq")
            nc.sync.dma_start(
                out=wq,
                in_=Wq[:, do * P:(do + 1) * P]
                .rearrange("(c p) n -> p c n", p=P).bitcast(F32R))
            for rb2 in range(M // NB):
                ps = mpsum.tile([P, NB], F32, tag="mps")
                for c in range(DC):
                    nc.tensor.matmul(
                        ps, wq[:, c, :],
                        imgT[c][:, rb2 * NB:(rb2 + 1) * NB],
                        start=(c == 0), stop=(c == DC - 1))
                nc.scalar.activation(qT[do][:, rb2 * NB:(rb2 + 1) * NB], ps,
                                     AF.Identity, bias=bq_t[:, do:do + 1],
                                     scale=1.0)
        wq_pool.release()
        imgT_pool.release()

        # ---- phase 4: S = qT.T @ kT (row-major), softmax fused per row ----
        kt_pool = tc.alloc_tile_pool(name="kt", bufs=20, side="left")
        attn_pool = tc.alloc_tile_pool(name="attn", bufs=1, side="right")
        attn = [attn_pool.tile([P, N_TXT], F32, tag=f"at{rb}", name=f"at{rb}")
                for rb in range(MB)]
        for tb in range(N_TXT // NB):
            kts = []
            for c in range(DC):
                kt = kt_pool.tile([P, NB], F32R, tag="kt", name=f"kt{tb}_{c}")
                nc.sync.dma_start(
                    out=kt,
                    in_=kT_gath[c * P:(c + 1) * P, tb * NB:(tb + 1) * NB]
                    .bitcast(F32R))
                kts.append(kt)
            for rb in range(MB):
                ps = mpsum.tile([P, NB], F32, tag="mps")
                for i in range(DC):
                    c = (2 * rb + i) % DC
                    anchor = nc.tensor.matmul(
                        ps, qT[c][:, rb * P:(rb + 1) * P], kts[c],
                        start=(i == 0), stop=(i == DC - 1))
                nc.vector.tensor_copy(attn[rb][:, tb * NB:(tb + 1) * NB], ps)
                if tb == N_TXT // NB - 1:
                    # row complete: exp(x - rowmax) in place, keep 1/rowsum
                    nc.vector.reduce_max(negmax[:, rb:rb + 1], attn[rb],
                                         axis=mybir.AxisListType.X,
                                         negate=True)
                    nc.scalar.activation(attn[rb], attn[rb], AF.Exp,
                                         bias=negmax[:, rb:rb + 1], scale=1.0,
                                         accum_out=rsum[:, rb:rb + 1])
                    nc.vector.reciprocal(recip[:, rb:rb + 1],
                                         rsum[:, rb:rb + 1])
        # v gather has ~150us of slack: hold it until scores finish so its
        # DMA traffic doesn't contend with wq/kt streaming
        ccv = nc.gpsimd.collective_compute(
            "AllGather", mybir.AluOpType.bypass,
            replica_groups=[list(range(NCORES))],
            ins=[v_slice.opt()], outs=[v_gath.opt()])
        tile.add_dep_helper(ccv.ins, anchor.ins,
                            reason="delay v gather past scores")
        kt_pool.release()
        qT_pool.release()

        # ---- phase 5: attnT[c][t, i] via PE transposes ----
        attnT_pool = tc.alloc_tile_pool(name="attnT", bufs=1, side="left")
        attnT = [attnT_pool.tile([P, M], F32R, tag=f"aT{c}", name=f"aT{c}")
                 for c in range(TC)]
        for rb in range(MB):
            for c in range(TC):
                ps = tpsum.tile([P, P], F32, tag="tps")
                nc.tensor.transpose(ps, attn[rb][:, c * P:(c + 1) * P], ident)
                nc.vector.tensor_copy(attnT[c][:, rb * P:(rb + 1) * P], ps)
        attn_pool.release()

        # ---- phase 6: avT[do][d, i] = (attn_unnorm @ v).T ----
        wo_pool = tc.alloc_tile_pool(name="wo", bufs=2, side="right")
        bop = tc.alloc_tile_pool(name="bop", bufs=1, side="right")
        bo_bc = bop.tile([P, D], F32, tag="bob")
        nc.gpsimd.dma_start(out=bo_bc, in_=bcast_ap(bo, D))
        ostage = tc.alloc_tile_pool(name="ost", bufs=2, side="right")
        avT_pool = tc.alloc_tile_pool(name="avT", bufs=1, side="right")
        vv_pool = tc.alloc_tile_pool(name="vv", bufs=2, side="right")
        avT = [avT_pool.tile([P, M], F32R, tag=f"av{c}", name=f"av{c}")
               for c in range(DC)]
        for do in range(DC):
            vv = vv_pool.tile([P, TC, P], F32R, tag="vv")
            nc.sync.dma_start(
                out=vv,
                in_=v_gath[:, do * P:(do + 1) * P]
                .rearrange("(c p) n -> p c n", p=P).bitcast(F32R))
            for rb2 in range(M // NB):
                ps = mpsum.tile([P, NB], F32, tag="mps")
                for c in range(TC):
                    nc.tensor.matmul(
                        ps, vv[:, c, :],
                        attnT[c][:, rb2 * NB:(rb2 + 1) * NB],
                        start=(c == 0), stop=(c == TC - 1))
                nc.vector.tensor_copy(avT[do][:, rb2 * NB:(rb2 + 1) * NB], ps)
        vv_pool.release()
        attnT_pool.release()

        # ---- phase 7: out = (avT.T @ Wo) * recip + bo (row-major) ----
        WB = NB // 2
        for ob in range(D // WB):
            wo = wo_pool.tile([P, DC, WB], F32R, tag="wo")
            nc.sync.dma_start(
                out=wo,
                in_=Wo[:, ob * WB:(ob + 1) * WB]
                .rearrange("(c p) n -> p c n", p=P).bitcast(F32R))
            for rb in range(MB):
                ps = mpsum.tile([P, WB], F32, tag="mps")
                for c in range(DC):
                    nc.tensor.matmul(
                        ps, avT[c][:, rb * P:(rb + 1) * P],
                        wo[:, c, :],
                        start=(c == 0), stop=(c == DC - 1))
                os_ = ostage.tile([P, WB], F32, tag="os")
                nc.scalar.activation(os_, ps, AF.Copy, bias=0.0,
                                     scale=recip[:, rb:rb + 1])
                nc.vector.tensor_add(os_, os_, bo_bc[:, ob * WB:(ob + 1) * WB])
                nc.sync.dma_start(
                    out=out[rb * P:(rb + 1) * P, ob * WB:(ob + 1) * WB],
                    in_=os_)
        avT_pool.release()
        ostage.release()
        bop.release()
        wo_pool.release()
        dram.release()
        mpsum.release()
        tpsum.release()
        consts.release()

    nc.finalize()
    return nc


_NC = None


def _get_nc():
    global _NC
    if _NC is None:
        _NC = build()
    return _NC


def make_in_maps(img, text_embedding, Wq, bq, Wk, bk, Wv, bv, Wo, bo):
    f = lambda x: np.ascontiguousarray(np.asarray(x, dtype=np.float32))
    img, text, Wk, bk = f(img), f(text_embedding), f(Wk), f(bk)
    base = dict(text_embedding=text, Wq=f(Wq), bq=f(bq), Wv=f(Wv), bv=f(bv),
                Wo=f(Wo), bo=f(bo))
    return [dict(base,
                 img=img[i * M:(i + 1) * M],
                 text_slice=text[i * SL:(i + 1) * SL].copy(),
                 Wk_slice=np.ascontiguousarray(Wk[:, i * SLD:(i + 1) * SLD]),
                 bk_slice=np.ascontiguousarray(bk[i * SLD:(i + 1) * SLD]))
            for i in range(NCORES)]


def kernel(img, text_embedding, Wq, bq, Wk, bk, Wv, bv, Wo, bo):
    nc = _get_nc()
    in_maps = make_in_maps(img, text_embedding, Wq, bq, Wk, bk, Wv, bv,
                           Wo, bo)
    res = run_bass_kernel_spmd(nc, in_maps, list(range(NCORES)))
    return np.concatenate([res.results[i]["out"] for i in range(NCORES)],
                          axis=0)


# revision 9
# speedup vs baseline: 1.0798x; 1.0798x over previous
"""Trainium2 Bass kernel for CrossAttention.

  q = img @ Wq + bq            [N_img, D]
  k = text @ Wk + bk           [N_txt, D]
  v = text @ Wv + bv           [N_txt, D]
  attn = softmax(q @ k.T, axis=1)
  out = (attn @ v) @ Wo + bo   [N_img, D]

Sharding: img rows split across 8 cores (row-parallel attention with
replicated weights). The k/v projections are 8-way sharded too — each
core computes a 1/8 slice (kT by output-feature rows, v by text rows)
and the full tensors are AllGathered while the img transpose + q
projection run. The per-core slice is selected host-side by feeding
each core its own Wk/bk column slice and text row slice, so the SPMD
program is identical across cores.

All matmuls run as float32r (full-rate truncated-fp32 streaming mode).
Softmax normalization is deferred: exp(s - max) stays unnormalized and
the 1/rowsum scale is applied per-partition during the final
output-projection PSUM eviction (exact because rows of attn scale
linearly through attn@v and @Wo; bo is added after the scale).

SBUF pools are stack-allocated per side; the overlapping phase chain
textT -> imgT -> qT -> attn -> attnT -> avT alternates sides so every
release is top-of-stack on its side.
"""

import os
import sys

sys.path.insert(0, "/opt/trn_rl_repo")

import numpy as np

import concourse.bass as bass
import concourse.tile as tile
from concourse import bacc, mybir
from concourse.bass_utils import run_bass_kernel_spmd
from concourse.masks import make_identity

F32 = mybir.dt.float32
F32R = mybir.dt.float32r
AF = mybir.ActivationFunctionType
P = 128
NCORES = 8

SMALL = bool(int(os.environ.get("BASS_SMALL", "0")))
if SMALL:
    N_IMG, D, N_TXT, DT, NB = 2048, 1024, 1024, 512, 256
else:
    N_IMG, D, N_TXT, DT, NB = 8192, 2048, 2048, 1024, 512

M = N_IMG // NCORES     # img rows per core
SL = N_TXT // NCORES    # text rows / kT feature-rows per core slice
SLD = D // NCORES       # Wk columns per core slice
DC = D // P             # chunks of the model dim
TC = N_TXT // P         # chunks of the text-token dim
DTC = DT // P           # chunks of the text-feature dim
MB = M // P             # img row blocks per core
SLB = SLD // P          # 128-blocks in the per-core kT slice
SLT = SL // P           # 128-blocks in the per-core text slice


def build():
    nc = bacc.Bacc("TRN2", target_bir_lowering=False, debug=False,
                   num_devices=NCORES)
    img = nc.declare_dram_parameter("img", [M, D], F32, isOutput=False)
    text = nc.declare_dram_parameter("text_embedding", [N_TXT, DT], F32,
                                     isOutput=False)
    text_sl = nc.declare_dram_parameter("text_slice", [SL, DT], F32,
                                        isOutput=False)
    Wq = nc.declare_dram_parameter("Wq", [D, D], F32, isOutput=False)
    bq = nc.declare_dram_parameter("bq", [D], F32, isOutput=False)
    Wk = nc.declare_dram_parameter("Wk_slice", [DT, SLD], F32, isOutput=False)
    bk = nc.declare_dram_parameter("bk_slice", [SLD], F32, isOutput=False)
    Wv = nc.declare_dram_parameter("Wv", [DT, D], F32, isOutput=False)
    bv = nc.declare_dram_parameter("bv", [D], F32, isOutput=False)
    Wo = nc.declare_dram_parameter("Wo", [D, D], F32, isOutput=False)
    bo = nc.declare_dram_parameter("bo", [D], F32, isOutput=False)
    out = nc.declare_dram_parameter("out", [M, D], F32, isOutput=True)

    def bcast_ap(handle, n):
        ap = handle.ap()
        return bass.AP(tensor=ap.tensor, offset=ap.offset,
                       ap=[[0, P]] + list(ap.ap))[:, :n]

    with tile.TileContext(nc) as tc:
        consts = tc.alloc_tile_pool(name="consts", bufs=1, side="left")
        tpsum = tc.alloc_tile_pool(name="tpsum", bufs=3, space="PSUM",
                                   side="left")
        mpsum = tc.alloc_tile_pool(name="mpsum", bufs=4, space="PSUM",
                                   side="left")
        dram = tc.alloc_tile_pool(name="dram", bufs=1, space="DRAM")

        kT_slice = dram.tile([SLD, N_TXT], F32, tag="kts")
        v_slice = dram.tile([SL, D], F32, tag="vs")
        kT_gath = dram.tile([D, N_TXT], F32, tag="ktg", addr_space="Shared")
        v_gath = dram.tile([N_TXT, D], F32, tag="vg", addr_space="Shared")

        ident = consts.tile([P, P], F32, tag="ident")
        make_identity(nc, ident)
        bq_t = consts.tile([P, DC], F32, tag="bqt")
        nc.gpsimd.dma_start(out=bq_t, in_=bq.rearrange("(c p) -> p c", p=P))
        bk_t = consts.tile([P, SLB], F32, tag="bkt")
        nc.gpsimd.dma_start(out=bk_t, in_=bk.rearrange("(c p) -> p c", p=P))
        negmax = consts.tile([P, MB], F32, tag="negmax")
        rsum = consts.tile([P, MB], F32, tag="rsum")
        recip = consts.tile([P, MB], F32, tag="recip")

        # ---- phase 0: textT[c][dt, t] = text.T, via PE transposes ----
        textT_pool = tc.alloc_tile_pool(name="textT", bufs=1, side="left")
        textTs_pool = tc.alloc_tile_pool(name="textTs", bufs=1, side="left")
        bvp = tc.alloc_tile_pool(name="bvp", bufs=1, side="left")
        bv_bc = bvp.tile([P, D], F32, tag="bvb")
        nc.gpsimd.dma_start(out=bv_bc, in_=bcast_ap(bv, D))
        wv_pool = tc.alloc_tile_pool(name="wv", bufs=3, side="left")
        vstage = tc.alloc_tile_pool(name="vstage", bufs=3, side="left")
        tstream = tc.alloc_tile_pool(name="tstream", bufs=6, side="left")
        textT = [textT_pool.tile([P, N_TXT], F32R, tag=f"tT{c}", name=f"tT{c}")
                 for c in range(DTC)]
        for j in range(TC):
            tr = tstream.tile([P, DT], F32, tag="trow")
            nc.sync.dma_start(out=tr, in_=text[j * P:(j + 1) * P, :])
            for c in range(DTC):
                ps = tpsum.tile([P, P], F32, tag="tps")
                nc.tensor.transpose(ps, tr[:, c * P:(c + 1) * P], ident)
                nc.vector.tensor_copy(textT[c][:, j * P:(j + 1) * P], ps)
        # textTs[c][dt, t_local]: transpose of this core's text row slice
        textTs = [textTs_pool.tile([P, SL], F32R, tag=f"tS{c}", name=f"tS{c}")
                  for c in range(DTC)]
        for j in range(SLT):
            tr = tstream.tile([P, DT], F32, tag="trow")
            nc.sync.dma_start(out=tr, in_=text_sl[j * P:(j + 1) * P, :])
            for c in range(DTC):
                ps = tpsum.tile([P, P], F32, tag="tps")
                nc.tensor.transpose(ps, tr[:, c * P:(c + 1) * P], ident)
                nc.vector.tensor_copy(textTs[c][:, j * P:(j + 1) * P], ps)
        tstream.release()

        # ---- phase 1a: kT slice = (text @ Wk_slice + bk_slice).T ----
        wk_pool = tc.alloc_tile_pool(name="wk", bufs=2, side="left")
        kstage = tc.alloc_tile_pool(name="kstage", bufs=3, side="left")
        for do in range(SLB):
            wk = wk_pool.tile([P, DTC, P], F32R, tag="wk")
            nc.sync.dma_start(
                out=wk,
                in_=Wk[:, do * P:(do + 1) * P]
                .rearrange("(c p) n -> p c n", p=P).bitcast(F32R))
            for tb in range(N_TXT // NB):
                ps = mpsum.tile([P, NB], F32, tag="mps")
                for c in range(DTC):
                    nc.tensor.matmul(
                        ps, wk[:, c, :],
                        textT[c][:, tb * NB:(tb + 1) * NB],
                        start=(c == 0), stop=(c == DTC - 1))
                ks = kstage.tile([P, NB], F32, tag="ks")
                nc.scalar.activation(ks, ps, AF.Identity,
                                     bias=bk_t[:, do:do + 1], scale=1.0)
                nc.sync.dma_start(
                    out=kT_slice[do * P:(do + 1) * P, tb * NB:(tb + 1) * NB],
                    in_=ks)
        nc.gpsimd.collective_compute(
            "AllGather", mybir.AluOpType.bypass,
            replica_groups=[list(range(NCORES))],
            ins=[kT_slice.opt()], outs=[kT_gath.opt()])
        kstage.release()
        wk_pool.release()

        # ---- phase 1b: v slice = text_slice @ Wv + bv (row-major) ----
        for ob in range(D // NB):
            wv = wv_pool.tile([P, DTC, NB], F32R, tag="wv")
            nc.sync.dma_start(
                out=wv,
                in_=Wv[:, ob * NB:(ob + 1) * NB]
                .rearrange("(c p) n -> p c n", p=P).bitcast(F32R))
            for j in range(SLT):
                ps = mpsum.tile([P, NB], F32, tag="mps")
                for c in range(DTC):
                    nc.tensor.matmul(
                        ps, textTs[c][:, j * P:(j + 1) * P],
                        wv[:, c, :],
                        start=(c == 0), stop=(c == DTC - 1))
                vs = vstage.tile([P, NB], F32, tag="vs")
                nc.vector.tensor_add(vs, ps, bv_bc[:, ob * NB:(ob + 1) * NB])
                nc.sync.dma_start(
                    out=v_slice[j * P:(j + 1) * P, ob * NB:(ob + 1) * NB],
                    in_=vs)
        vstage.release()
        wv_pool.release()
        bvp.release()
        textTs_pool.release()
        textT_pool.release()

        # ---- phase 2: imgT[c][d, i] = img.T ----
        imgT_pool = tc.alloc_tile_pool(name="imgT", bufs=1, side="right")
        istream = tc.alloc_tile_pool(name="istream", bufs=8, side="right")
        imgT = [imgT_pool.tile([P, M], F32R, tag=f"iT{c}", name=f"iT{c}")
                for c in range(DC)]
        for rb in range(MB):
            ir = istream.tile([P, D], F32, tag="irow")
            nc.sync.dma_start(out=ir, in_=img[rb * P:(rb + 1) * P, :])
            for c in range(DC):
                ps = tpsum.tile([P, P], F32, tag="tps")
                nc.tensor.transpose(ps, ir[:, c * P:(c + 1) * P], ident)
                nc.vector.tensor_copy(imgT[c][:, rb * P:(rb + 1) * P], ps)
        istream.release()

        # ---- phase 3: qT[do][d_out, i] = (img @ Wq + bq).T, SBUF-resident ----
        qT_pool = tc.alloc_tile_pool(name="qT", bufs=1, side="left")
        wq_pool = tc.alloc_tile_pool(name="wq", bufs=4, side="left")
        qT = [qT_pool.tile([P, M], F32R, tag=f"qT{c}", name=f"qT{c}")
              for c in range(DC)]
        for do in range(DC):
            wq = wq_pool.tile([P, DC, P], F32R, tag="wq")
            nc.sync.dma_start(
                out=wq,
                in_=Wq[:, do * P:(do + 1) * P]
                .rearrange("(c p) n -> p c n", p=P).bitcast(F32R))
            for rb2 in range(M // NB):
                ps = mpsum.tile([P, NB], F32, tag="mps")
                for c in range(DC):
                    nc.tensor.matmul(
                        ps, wq[:, c, :],
                        imgT[c][:, rb2 * NB:(rb2 + 1) * NB],
                        start=(c == 0), stop=(c == DC - 1))
                nc.scalar.activation(qT[do][:, rb2 * NB:(rb2 + 1) * NB], ps,
                                     AF.Identity, bias=bq_t[:, do:do + 1],
                                     scale=1.0)
        wq_pool.release()
        imgT_pool.release()

        # ---- phase 4: S = qT.T @ kT (row-major), softmax fused per row ----
        kt_pool = tc.alloc_tile_pool(name="kt", bufs=24, side="left")
        attn_pool = tc.alloc_tile_pool(name="attn", bufs=1, side="right")
        attn = [attn_pool.tile([P, N_TXT], F32, tag=f"at{rb}", name=f"at{rb}")
                for rb in range(MB)]
        for tb in range(N_TXT // NB):
            kts = []
            for c in range(DC):
                kt = kt_pool.tile([P, NB], F32R, tag="kt", name=f"kt{tb}_{c}")
                nc.sync.dma_start(
                    out=kt,
                    in_=kT_gath[c * P:(c + 1) * P, tb * NB:(tb + 1) * NB]
                    .bitcast(F32R))
                kts.append(kt)
            for rb in range(MB):
                ps = mpsum.tile([P, NB], F32, tag="mps")
                for i in range(DC):
                    c = (2 * rb + i) % DC
                    mm = nc.tensor.matmul(
                        ps, qT[c][:, rb * P:(rb + 1) * P], kts[c],
                        start=(i == 0), stop=(i == DC - 1))
                if tb == 1 and rb == MB - 1:
                    anchor = mm
                nc.vector.tensor_copy(attn[rb][:, tb * NB:(tb + 1) * NB], ps)
                if tb == N_TXT // NB - 1:
                    # row complete: exp(x - rowmax) in place, keep 1/rowsum
                    nc.vector.reduce_max(negmax[:, rb:rb + 1], attn[rb],
                                         axis=mybir.AxisListType.X,
                                         negate=True)
                    nc.scalar.activation(attn[rb], attn[rb], AF.Exp,
                                         bias=negmax[:, rb:rb + 1], scale=1.0,
                                         accum_out=rsum[:, rb:rb + 1])
                    nc.vector.reciprocal(recip[:, rb:rb + 1],
                                         rsum[:, rb:rb + 1])
        # v gather has ~150us of slack: hold it until scores finish so its
        # DMA traffic doesn't contend with wq/kt streaming
        ccv = nc.gpsimd.collective_compute(
            "AllGather", mybir.AluOpType.bypass,
            replica_groups=[list(range(NCORES))],
            ins=[v_slice.opt()], outs=[v_gath.opt()])
        tile.add_dep_helper(ccv.ins, anchor.ins,
                            reason="delay v gather past scores")
        kt_pool.release()
        qT_pool.release()

        # ---- phase 5: attnT[c][t, i] via PE transposes ----
        attnT_pool = tc.alloc_tile_pool(name="attnT", bufs=1, side="left")
        attnT = [attnT_pool.tile([P, M], F32R, tag=f"aT{c}", name=f"aT{c}")
                 for c in range(TC)]
        for rb in range(MB):
            for c in range(TC):
                ps = tpsum.tile([P, P], F32, tag="tps")
                nc.tensor.transpose(ps, attn[rb][:, c * P:(c + 1) * P], ident)
                nc.vector.tensor_copy(attnT[c][:, rb * P:(rb + 1) * P], ps)
        attn_pool.release()

        # ---- phase 6: avT[do][d, i] = (attn_unnorm @ v).T ----
        wo_pool = tc.alloc_tile_pool(name="wo", bufs=2, side="right")
        bop = tc.alloc_tile_pool(name="bop", bufs=1, side="right")
        bo_bc = bop.tile([P, D], F32, tag="bob")
        nc.gpsimd.dma_start(out=bo_bc, in_=bcast_ap(bo, D))
        ostage = tc.alloc_tile_pool(name="ost", bufs=2, side="right")
        avT_pool = tc.alloc_tile_pool(name="avT", bufs=1, side="right")
        vv_pool = tc.alloc_tile_pool(name="vv", bufs=2, side="right")
        avT = [avT_pool.tile([P, M], F32R, tag=f"av{c}", name=f"av{c}")
               for c in range(DC)]
        for do in range(DC):
            vv = vv_pool.tile([P, TC, P], F32R, tag="vv")
            nc.sync.dma_start(
                out=vv,
                in_=v_gath[:, do * P:(do + 1) * P]
                .rearrange("(c p) n -> p c n", p=P).bitcast(F32R))
            for rb2 in range(M // NB):
                ps = mpsum.tile([P, NB], F32, tag="mps")
                for c in range(TC):
                    nc.tensor.matmul(
                        ps, vv[:, c, :],
                        attnT[c][:, rb2 * NB:(rb2 + 1) * NB],
                        start=(c == 0), stop=(c == TC - 1))
                nc.vector.tensor_copy(avT[do][:, rb2 * NB:(rb2 + 1) * NB], ps)
        vv_pool.release()
        attnT_pool.release()

        # ---- phase 7: out = (avT.T @ Wo) * recip + bo (row-major) ----
        WB = NB // 2
        for ob in range(D // WB):
            wo = wo_pool.tile([P, DC, WB], F32R, tag="wo")
            nc.sync.dma_start(
                out=wo,
                in_=Wo[:, ob * WB:(ob + 1) * WB]
                .rearrange("(c p) n -> p c n", p=P).bitcast(F32R))
            for rb in range(MB):
                ps = mpsum.tile([P, WB], F32, tag="mps")
                for c in range(DC):
                    nc.tensor.matmul(
                        ps, avT[c][:, rb * P:(rb + 1) * P],
                        wo[:, c, :],
                        start=(c == 0), stop=(c == DC - 1))
                os_ = ostage.tile([P, WB], F32, tag="os")
                nc.scalar.activation(os_, ps, AF.Copy, bias=0.0,
                                     scale=recip[:, rb:rb + 1])
                nc.vector.tensor_add(os_, os_, bo_bc[:, ob * WB:(ob + 1) * WB])
                nc.sync.dma_start(
                    out=out[rb * P:(rb + 1) * P, ob * WB:(ob + 1) * WB],
                    in_=os_)
        avT_pool.release()
        ostage.release()
        bop.release()
        wo_pool.release()
        dram.release()
        mpsum.release()
        tpsum.release()
        consts.release()

    nc.finalize()
    return nc


_NC = None


def _get_nc():
    global _NC
    if _NC is None:
        _NC = build()
    return _NC


def make_in_maps(img, text_embedding, Wq, bq, Wk, bk, Wv, bv, Wo, bo):
    f = lambda x: np.ascontiguousarray(np.asarray(x, dtype=np.float32))
    img, text, Wk, bk = f(img), f(text_embedding), f(Wk), f(bk)
    base = dict(text_embedding=text, Wq=f(Wq), bq=f(bq), Wv=f(Wv), bv=f(bv),
                Wo=f(Wo), bo=f(bo))
    return [dict(base,
                 img=img[i * M:(i + 1) * M],
                 text_slice=text[i * SL:(i + 1) * SL].copy(),
                 Wk_slice=np.ascontiguousarray(Wk[:, i * SLD:(i + 1) * SLD]),
                 bk_slice=np.ascontiguousarray(bk[i * SLD:(i + 1) * SLD]))
            for i in range(NCORES)]


def kernel(img, text_embedding, Wq, bq, Wk, bk, Wv, bv, Wo, bo):
    nc = _get_nc()
    in_maps = make_in_maps(img, text_embedding, Wq, bq, Wk, bk, Wv, bv,
                           Wo, bo)
    res = run_bass_kernel_spmd(nc, in_maps, list(range(NCORES)))
    return np.concatenate([res.results[i]["out"] for i in range(NCORES)],
                          axis=0)
